# revision 26
# baseline (speedup 1.0000x reference)
"""Distributed Trainium2 kernel for a 2-relation GNN message-passing layer.

agg = x @ W_self.T + sum_r scatter_add(x[src_r] @ W_r.T, tgt_r)

Strategy (8 NeuronCores, SPMD, no collectives):
- Targets sharded: core c owns rows [c*62500, (c+1)*62500) of the output.
- x (bf16) replicated to every core as 16 bucket tensors of 31250 rows
  (dma_gather uses int16 indices, so gather tables stay < 32768 rows).
- Targets split into 8 regions of 16 windows (512 targets each).  Staging
  table per region holds BOTH relations (2 rel x 16 buckets x 640 rows =
  20480 < 32768), so one phase-B gather serves a window-pair across both
  relations.
- Phase A: per (rel, bucket, region-quad) one dma_gather of 4x640 = 2560
  bucket-local rows, written to the 4 region staging tables as big
  contiguous partition-major descriptors.  64 gathers total.
- Phase B: per window-pair one dma_gather of 2560 rows from the region
  staging table in target-sorted order (62 gathers).  Per window:
  one-hot scatter matmuls (5 blocks x narrow ranges per rel) into a
  PSUM agg bank gps[f, t], then transposed GEMMs
  out[o, t] = Wself.T @ xT + W0.T @ agg0 + W1.T @ agg1, each one matmul
  streaming 512 columns, accumulated in a second PSUM bank.
- All instruction shapes identical across cores (SPMD); per-core variation
  lives in the index tensors.  Pad slots gather row 0 (valid, finite) and
  carry one-hot target value -1 (matches nothing -> contributes zero).
"""
import os
import sys
import types

import numpy as np

sys.path.insert(0, "/opt/trn_rl_repo")

N = 500_000
D = 128
NUM_REL = 2
NCORE = 8
TPC = N // NCORE            # 62500 targets per core
NB = 16                     # src buckets
BROWS = N // NB             # 31250 rows per bucket
WIN = 512                   # aggregation window (one PSUM bank: 512 f32)
NWIN = (TPC + WIN - 1) // WIN          # 123 windows
WPR = 16                    # windows per region
NREG = (NWIN + WPR - 1) // WPR         # 8 regions
SUBCAP = 640                # slots per (rel, bucket, region) staging chunk
BCAP = 640                  # slots per (rel, window) in phase B
STAGE_ROWS = NUM_REL * NB * SUBCAP     # 20480 rows per region table
NPAIR = (NWIN + 1) // 2                # 62 window-pairs
PAIR_SLOTS = 4 * BCAP                  # 2560 slots per pair
GSLOT = 1024                # slots per dma_gather (64-desc packet limit)
RB_SLOTS = NREG * SUBCAP               # 5120 slots per (rel, bucket)
A_SLOTS = NUM_REL * NB * RB_SLOTS          # 163840
NGROUP = (NPAIR + 1) // 2              # 31 groups of 2 pairs
GROUP_SLOTS = 2 * PAIR_SLOTS           # 5120 slots per phase-B group
B_SLOTS = NGROUP * GROUP_SLOTS             # 158720
TPCP = NGROUP * 4 * WIN     # 63488 padded target count (full groups)
NQUEUE = 4


def _register_profile_hook():
    if "antenv.axon_hooks" in sys.modules:
        return
    mod = types.ModuleType("antenv.axon_hooks")
    state = {"h": None}
    mod.set_axon_ntff_profile_hook = lambda h: state.__setitem__("h", h)
    mod.get_axon_ntff_profile_hook = lambda: state["h"]
    sys.modules["antenv.axon_hooks"] = mod
    try:
        from trn_agent_boot.trn_boot import _ntff_profile_via_ctypes
        mod.set_axon_ntff_profile_hook(
            _ntff_profile_via_ctypes("/opt/axon/libaxon_pjrt.so"))
    except Exception:
        pass


def _pack_core(edge_indices, core):
    """Build per-core index tensors.

    Returns:
      agidx [A_SLOTS] int16   phase-A gather idxs (bucket-local src), 0 pad
      bgidx [B_SLOTS] int16   phase-B gather idxs (region staging row), 0 pad
      btv   [B_SLOTS] f32     phase-B window-local target value, -1 pad
    """
    lo = core * TPC
    agidx = np.zeros(A_SLOTS, dtype=np.int16)
    bgidx = np.zeros(B_SLOTS, dtype=np.int16)
    btv = np.full(B_SLOTS, -1.0, dtype=np.float32)

    for r in range(NUM_REL):
        src = np.asarray(edge_indices[r, 0])
        tgt = np.asarray(edge_indices[r, 1])
        m = (tgt >= lo) & (tgt < lo + TPC)
        s = src[m].astype(np.int64)
        t = (tgt[m] - lo).astype(np.int64)
        w = t // WIN
        reg = np.minimum(w // WPR, NREG - 1)
        b = s // BROWS

        # ---- phase A placement: group by (region, bucket) ----
        order_a = np.lexsort((t, b, reg))
        rs, bs = reg[order_a], b[order_a]
        grp = rs * NB + bs
        # rank within group
        change = np.empty(grp.shape[0], dtype=bool)
        if grp.shape[0]:
            change[0] = True
            change[1:] = grp[1:] != grp[:-1]
        starts = np.nonzero(change)[0]
        gid = np.cumsum(change) - 1
        rank = np.arange(grp.shape[0]) - starts[gid]
        counts = np.bincount(grp, minlength=NREG * NB)
        if counts.max(initial=0) > SUBCAP:
            raise RuntimeError(
                f"phase-A overflow r{r} core{core}: {counts.max()}")
        # slot space per (rel, bucket): regions in order, SUBCAP each
        aslot = (r * NB + bs) * RB_SLOTS + rs * SUBCAP + rank
        agidx[aslot] = (s[order_a] - bs * BROWS).astype(np.int16)
        # staging row in region table (partition-major permuted within chunk)
        nblk = SUBCAP // 128
        soff = (r * NB + bs) * SUBCAP
        stage_row_sorted = soff + (rank % 128) * nblk + rank // 128
        stage_row = np.empty(t.shape[0], dtype=np.int64)
        stage_row[order_a] = stage_row_sorted

        # ---- phase B placement: group by (window), target-sorted ----
        order_b = np.lexsort((t,))
        wsB, tsB = w[order_b], t[order_b]
        rowsB = stage_row[order_b]
        changeB = np.empty(wsB.shape[0], dtype=bool)
        if wsB.shape[0]:
            changeB[0] = True
            changeB[1:] = wsB[1:] != wsB[:-1]
        startsB = np.nonzero(changeB)[0]
        gidB = np.cumsum(changeB) - 1
        rankB = np.arange(wsB.shape[0]) - startsB[gidB]
        countsB = np.bincount(wsB, minlength=NWIN)
        if countsB.max(initial=0) > BCAP:
            raise RuntimeError(
                f"phase-B overflow r{r} core{core}: {countsB.max()}")
        pair = wsB // 2
        subw = wsB % 2
        bslot = pair * PAIR_SLOTS + (subw * NUM_REL + r) * BCAP + rankB
        bgidx[bslot] = rowsB.astype(np.int16)
        btv[bslot] = (tsB - wsB * WIN).astype(np.float32)
    return agidx, bgidx, btv


def _block_ranges(packs):
    """Per global 128-slot block: union across cores of the active
    window-local target range [lo, hi).  All-pad blocks get (0, 1)."""
    nblocks = B_SLOTS // 128
    btvs = np.stack([p[2] for p in packs])            # [NCORE, B_SLOTS]
    btvs = btvs.reshape(NCORE, nblocks, 128)
    valid = btvs >= 0
    lob = np.where(valid, btvs, np.inf).min(axis=(0, 2))
    hib = np.where(valid, btvs, -np.inf).max(axis=(0, 2))
    ranges = []
    for j in range(nblocks):
        if np.isfinite(lob[j]):
            ranges.append((int(lob[j]), int(hib[j]) + 1))
        else:
            ranges.append((0, 1))
    return ranges


def _wrap16(idx_flat):
    n = idx_flat.shape[0]
    a = idx_flat.reshape(n // 16, 16).T
    return np.tile(a, (8, 1)).copy()


def _slotmaj(v):
    n = v.shape[0]
    return np.ascontiguousarray(v.reshape(n // 128, 128).T)


def _build_program(ranges, queue_map=None):
    import concourse.bacc as bacc
    import concourse.tile as tile
    from concourse import mybir

    nc = bacc.Bacc("TRN2", debug=False, num_swdge_queues=NQUEUE)
    dt = mybir.dt

    xb = [nc.dram_tensor(f"xb{k}", [BROWS, D], dt.bfloat16, kind="ExternalInput")
          for k in range(NB)]
    xto = nc.dram_tensor("xto", [D, TPCP], dt.bfloat16, kind="ExternalInput")
    wt = nc.dram_tensor("wt", [D, 3 * D], dt.bfloat16, kind="ExternalInput")
    agidx_d = nc.dram_tensor("agidx", [128, A_SLOTS // 16], dt.int16,
                             kind="ExternalInput")
    bgidx_d = nc.dram_tensor("bgidx", [128, B_SLOTS // 16], dt.int16,
                             kind="ExternalInput")
    btv_d = nc.dram_tensor("btv", [128, B_SLOTS // 128], dt.float16,
                           kind="ExternalInput")
    iota_d = nc.dram_tensor("iota", [128, WIN], dt.float16, kind="ExternalInput")
    stage = [nc.dram_tensor(f"stage{h}", [STAGE_ROWS, D], dt.bfloat16,
                            kind="ExternalOutput") for h in range(NREG)]
    out_d = nc.dram_tensor("out", [128, TPCP], dt.bfloat16,
                           kind="ExternalOutput")

    nblk_sub = SUBCAP // 128            # 5
    nblk_g = GSLOT // 128               # 8 blocks per gather

    with tile.TileContext(nc) as tc:
        with (
            tc.tile_pool(name="const", bufs=1) as cpool,
            tc.tile_pool(name="ag", bufs=12) as agpool,
            tc.tile_pool(name="bg", bufs=6) as bgpool,
            tc.tile_pool(name="oh", bufs=16) as ohpool,
            tc.tile_pool(name="gsb", bufs=10) as gsbpool,
            tc.tile_pool(name="xt", bufs=4) as xtpool,
            tc.tile_pool(name="osb", bufs=4) as osbpool,
            tc.tile_pool(name="psA", bufs=4, space="PSUM") as psA,
            tc.tile_pool(name="psB", bufs=4, space="PSUM") as psB,
        ):
            wt_sb = cpool.tile([D, 3 * D], dt.bfloat16)
            nc.sync.dma_start(wt_sb[:], wt[:])
            iota_sb = cpool.tile([128, WIN], dt.float16)
            nc.sync.dma_start(iota_sb[:], iota_d[:])
            agidx_sb = cpool.tile([128, A_SLOTS // 16], dt.int16)
            nc.sync.dma_start(agidx_sb[:], agidx_d[:])
            bgidx_sb = cpool.tile([128, B_SLOTS // 16], dt.int16)
            nc.sync.dma_start(bgidx_sb[:], bgidx_d[:])
            btv_sb = cpool.tile([128, B_SLOTS // 128], dt.float16)
            nc.sync.dma_start(btv_sb[:], btv_d[:])

            gcount = [0]
            gather_names = []

            def gq():
                # queue for the next SWDGE gather, from the two-pass map
                q = queue_map[gcount[0]] if queue_map else 0
                gcount[0] += 1
                return q

            def emit_a_level(g, r, b):
                # one 1024-slot gather of (rel, bucket) slot-range
                # [1024g, 1024(g+1)), written to the region tables it spans
                ga = agpool.tile([128, nblk_g, D], dt.bfloat16, tag="ag")
                base = (r * NB + b) * RB_SLOTS
                fb = base + g * GSLOT
                gi = nc.gpsimd.dma_gather(
                    ga[:], xb[b][:],
                    agidx_sb[:, fb // 16:(fb + GSLOT) // 16],
                    GSLOT, GSLOT, D,
                    queue_num=gq(),
                )
                gather_names.append(gi.ins.name)
                soff = (r * NB + b) * SUBCAP
                s0 = g * GSLOT
                s1 = s0 + GSLOT
                h0 = s0 // SUBCAP
                h1 = (s1 - 1) // SUBCAP
                for h in range(h0, h1 + 1):
                    c0 = max(s0, h * SUBCAP) - h * SUBCAP      # chunk-local
                    c1 = min(s1, (h + 1) * SUBCAP) - h * SUBCAP
                    t0b = (h * SUBCAP + c0 - s0) // 128        # tile block
                    nc.sync.dma_start(
                        stage[h][soff:soff + SUBCAP, :].rearrange(
                            "(p j) o -> p j o",
                            p=128)[:, c0 // 128:c1 // 128, :],
                        ga[:, t0b:t0b + (c1 - c0) // 128, :],
                    )

            def emit_b_group(G):
                h = (4 * G) // WPR          # region of this group's windows
                gb = bgpool.tile([128, GROUP_SLOTS // 128, D], dt.bfloat16,
                                 tag="bg")
                goff = G * GROUP_SLOTS
                for g in range(GROUP_SLOTS // GSLOT):
                    fb = goff + g * GSLOT
                    gi = nc.gpsimd.dma_gather(
                        gb[:, g * nblk_g:(g + 1) * nblk_g, :], stage[h][:],
                        bgidx_sb[:, fb // 16:(fb + GSLOT) // 16],
                        GSLOT, GSLOT, D,
                        queue_num=gq(),
                    )
                    gather_names.append(gi.ins.name)
                xt_t = xtpool.tile([D, 4 * WIN], dt.bfloat16, tag="xt")
                nc.sync.dma_start(xt_t[:], xto[:, G * 4 * WIN:(G + 1) * 4 * WIN])
                osb = osbpool.tile([128, 4 * WIN], dt.bfloat16, tag="osb")
                # scatter phase for all 4 windows first, then the 12 GEMMs
                # grouped by weight so consecutive matmuls share lhsT
                gsbs = {}
                for wl in range(4):
                    for r in range(NUM_REL):
                        gps = psA.tile([128, WIN], dt.float32, tag="psA")
                        nc.vector.memset(gps[:], 0.0)
                        jbase = (wl * NUM_REL + r) * nblk_sub
                        for j in range(nblk_sub):
                            bcol = goff // 128 + jbase + j
                            lo, hi = ranges[bcol]
                            oh = ohpool.tile([128, WIN], dt.bfloat16, tag="oh")
                            nc.vector.tensor_tensor(
                                out=oh[:, :hi - lo],
                                in0=btv_sb[:, bcol:bcol + 1]
                                    .to_broadcast([128, hi - lo]),
                                in1=iota_sb[:, lo:hi],
                                op=mybir.AluOpType.is_equal,
                            )
                            nc.tensor.matmul(
                                gps[:, lo:hi],
                                gb[:, jbase + j, :],
                                oh[:, :hi - lo],
                                start=False, stop=(j == nblk_sub - 1),
                                skip_group_check=True,
                            )
                        gsb = gsbpool.tile([128, WIN], dt.bfloat16, tag="gsb")
                        nc.scalar.copy(out=gsb[:], in_=gps[:])
                        gsbs[(wl, r)] = gsb
                outps = {}
                for wl in range(4):
                    outp = psB.tile([128, WIN], dt.float32, tag="psB")
                    outps[wl] = outp
                    nc.tensor.matmul(
                        outp[:], wt_sb[:, 0:D], xt_t[:, wl * WIN:(wl + 1) * WIN],
                        start=True, stop=False,
                    )
                for r in range(NUM_REL):
                    for wl in range(4):
                        nc.tensor.matmul(
                            outps[wl][:], wt_sb[:, (1 + r) * D:(2 + r) * D],
                            gsbs[(wl, r)][:],
                            start=False, stop=(r == NUM_REL - 1),
                        )
                for wl in range(4):
                    nc.scalar.copy(out=osb[:, wl * WIN:(wl + 1) * WIN],
                                   in_=outps[wl][:])
                nc.sync.dma_start(
                    out_d[:, G * 4 * WIN:(G + 1) * 4 * WIN], osb[:])

            # Emission: A levels feed regions in order; B groups of region h
            # need A levels 0..ceil(((h+1)*640)/1024)-1.  Interleave so the
            # SWDGE queues stay busy while PE/DVE consume earlier regions.
            _phase = os.environ.get("KPHASE", "")
            rb = [(r, b) for r in range(NUM_REL) for b in range(NB)]
            nlv = RB_SLOTS // GSLOT                 # 5 A levels
            # B group G (windows 4G..4G+3) is in region G//4; region h is
            # fully staged after A level lv_need[h].  Emit levels 0-1 up
            # front, then interleave later levels' gathers between the B
            # groups they do NOT gate, so the Pool queues and PE both stay
            # busy instead of alternating in bursts.
            lv_need = [((h + 1) * SUBCAP - 1) // GSLOT for h in range(NREG)]
            ready_after = {lv: [] for lv in range(nlv)}
            for G in range(NGROUP):
                ready_after[lv_need[G // 4]].append(G)

            for (r, b) in rb:
                emit_a_level(0, r, b)
            for (r, b) in rb:
                emit_a_level(1, r, b)
            pending_b = ready_after[0] + ready_after[1]
            for lv in range(2, nlv):
                nb_ = len(pending_b)
                na = len(rb)
                k = 0
                for i, G in enumerate(pending_b):
                    if _phase != "A":
                        emit_b_group(G)
                    while k * nb_ < (i + 1) * na:
                        emit_a_level(lv, *rb[k])
                        k += 1
                while k < na:
                    emit_a_level(lv, *rb[k])
                    k += 1
                pending_b = ready_after[lv]
            for G in pending_b:
                if _phase != "A":
                    emit_b_group(G)
    nc.compile()
    return nc, gather_names


def _pool_dma_sched_order(nc):
    """Names of Pool-engine DMA instructions in scheduled (block) order --
    the order the Tile sem-assignment pass walks, which fixes each
    instruction's DMASW sem lane (lane = position % 8)."""
    from concourse import mybir
    order = []
    for f in nc.m.functions:
        for blk in f.blocks:
            for inst in blk.instructions:
                if (getattr(inst, 'engine', None) == mybir.EngineType.Pool
                        and getattr(inst, 'queue_num', None) is not None):
                    order.append(inst.name)
    return order


def _build_with_queues(ranges):
    """Two-pass build: DMASW sem lanes are assigned by scheduled position
    (mod 8), and a sem lane is locked to one SWDGE queue.  queue =
    scheduled_position % NQUEUE satisfies the lock for every lane.  The
    schedule can shift slightly when queue numbers change, so iterate to a
    fixpoint (falling back to single-queue if it doesn't settle)."""
    queue_map = None
    for attempt in range(4):
        nc, names = _build_program(ranges, queue_map)
        sched = {n: i for i, n in enumerate(_pool_dma_sched_order(nc))}
        want = {gi: sched[n] % NQUEUE for gi, n in enumerate(names)}
        if queue_map == want:
            return nc
        queue_map = want
    # last build used the latest map; verify consistency, else rebuild q0
    nc, names = _build_program(ranges, queue_map)
    sched = {n: i for i, n in enumerate(_pool_dma_sched_order(nc))}
    ok = all(sched[n] % NQUEUE == queue_map[gi]
             for gi, n in enumerate(names))
    if ok:
        return nc
    nc, _ = _build_program(ranges, {gi: 0 for gi in range(len(names))})
    return nc


_NC_CACHE = {}


def kernel(x, W0, W1, W_self, edge_indices):
    import ml_dtypes
    from concourse import bass_utils
    from concourse.bass_utils import run_bass_kernel_spmd

    _register_profile_hook()
    bass_utils.upload_artifacts = lambda tmpdir: "local://" + tmpdir

    x = np.asarray(x)
    W0 = np.asarray(W0)
    W1 = np.asarray(W1)
    W_self = np.asarray(W_self)
    edge_indices = np.asarray(edge_indices)

    bf16 = ml_dtypes.bfloat16
    x16 = x.astype(bf16)
    xbufs = [np.ascontiguousarray(x16[k * BROWS:(k + 1) * BROWS])
             for k in range(NB)]
    wt = np.concatenate([W_self.T, W0.T, W1.T], axis=1).astype(bf16)
    iota = np.tile(np.arange(WIN, dtype=np.float16), (128, 1))

    packs = [_pack_core(edge_indices, c) for c in range(NCORE)]
    ranges = _block_ranges(packs)
    if "nc" not in _NC_CACHE:
        _NC_CACHE["nc"] = _build_with_queues(ranges)
    nc = _NC_CACHE["nc"]

    in_maps = []
    for c in range(NCORE):
        agidx, bgidx, btv = packs[c]
        im = {f"xb{k}": xbufs[k] for k in range(NB)}
        xt = np.zeros((D, TPCP), dtype=bf16)
        xt[:, :TPC] = x16[c * TPC:(c + 1) * TPC].T
        im["xto"] = xt
        im["wt"] = wt
        im["agidx"] = _wrap16(agidx)
        im["bgidx"] = _wrap16(bgidx)
        im["btv"] = _slotmaj(btv.astype(np.float16))
        im["iota"] = iota
        in_maps.append(im)

    trace = os.environ.get("KBENCH_TRACE", "0") == "1"
    res = run_bass_kernel_spmd(nc, in_maps, core_ids=list(range(NCORE)),
                               trace=trace)
    if trace:
        print("HW exec time:", res.exec_time_ns, "ns")
        _NC_CACHE["exec_time_ns"] = res.exec_time_ns

    out = np.empty((N, D), dtype=np.float32)
    for c in range(NCORE):
        o = np.asarray(res.results[c]["out"])          # [128, TPCP] bf16
        out[c * TPC:(c + 1) * TPC] = o[:, :TPC].T.astype(np.float32)
    return out


# revision 28
# speedup vs baseline: 1.2197x; 1.2197x over previous
"""Distributed Trainium2 kernel for a 2-relation GNN message-passing layer.

agg = x @ W_self.T + sum_r scatter_add(x[src_r] @ W_r.T, tgt_r)

Strategy (8 NeuronCores, SPMD, no collectives):
- Targets sharded: core c owns rows [c*62500, (c+1)*62500) of the output.
- x (bf16) replicated to every core as 16 bucket tensors of 31250 rows
  (dma_gather uses int16 indices, so gather tables stay < 32768 rows).
- Targets split into 8 regions of 16 windows (512 targets each).  Staging
  table per region holds BOTH relations (2 rel x 16 buckets x 640 rows =
  20480 < 32768), so one phase-B gather serves a window-pair across both
  relations.
- Phase A: per (rel, bucket, region-quad) one dma_gather of 4x640 = 2560
  bucket-local rows, written to the 4 region staging tables as big
  contiguous partition-major descriptors.  64 gathers total.
- Phase B: per window-pair one dma_gather of 2560 rows from the region
  staging table in target-sorted order (62 gathers).  Per window:
  one-hot scatter matmuls (5 blocks x narrow ranges per rel) into a
  PSUM agg bank gps[f, t], then transposed GEMMs
  out[o, t] = Wself.T @ xT + W0.T @ agg0 + W1.T @ agg1, each one matmul
  streaming 512 columns, accumulated in a second PSUM bank.
- All instruction shapes identical across cores (SPMD); per-core variation
  lives in the index tensors.  Pad slots gather row 0 (valid, finite) and
  carry one-hot target value -1 (matches nothing -> contributes zero).
"""
import os
import sys
import types

import numpy as np

sys.path.insert(0, "/opt/trn_rl_repo")

N = 500_000
D = 128
NUM_REL = 2
NCORE = 8
TPC = N // NCORE            # 62500 targets per core
NB = 16                     # src buckets
BROWS = N // NB             # 31250 rows per bucket
WIN = 512                   # aggregation window (one PSUM bank: 512 f32)
NWIN = (TPC + WIN - 1) // WIN          # 123 windows
WPR = 16                    # windows per region
NREG = (NWIN + WPR - 1) // WPR         # 8 regions
SUBCAP = 640                # slots per (rel, bucket, region) staging chunk
BCAP = 640                  # slots per (rel, window) in phase B
STAGE_ROWS = NUM_REL * NB * SUBCAP     # 20480 rows per region table
NPAIR = (NWIN + 1) // 2                # 62 window-pairs
PAIR_SLOTS = 4 * BCAP                  # 2560 slots per pair
GSLOT = 1024                # slots per dma_gather (64-desc packet limit)
RB_SLOTS = NREG * SUBCAP               # 5120 slots per (rel, bucket)
A_SLOTS = NUM_REL * NB * RB_SLOTS          # 163840
NGROUP = (NPAIR + 1) // 2              # 31 groups of 2 pairs
GROUP_SLOTS = 2 * PAIR_SLOTS           # 5120 slots per phase-B group
B_SLOTS = NGROUP * GROUP_SLOTS             # 158720
TPCP = NGROUP * 4 * WIN     # 63488 padded target count (full groups)
NQUEUE = 4


def _register_profile_hook():
    if "antenv.axon_hooks" in sys.modules:
        return
    mod = types.ModuleType("antenv.axon_hooks")
    state = {"h": None}
    mod.set_axon_ntff_profile_hook = lambda h: state.__setitem__("h", h)
    mod.get_axon_ntff_profile_hook = lambda: state["h"]
    sys.modules["antenv.axon_hooks"] = mod
    try:
        from trn_agent_boot.trn_boot import _ntff_profile_via_ctypes
        mod.set_axon_ntff_profile_hook(
            _ntff_profile_via_ctypes("/opt/axon/libaxon_pjrt.so"))
    except Exception:
        pass


def _pack_core(edge_indices, core):
    """Build per-core index tensors.

    Returns:
      agidx [A_SLOTS] int16   phase-A gather idxs (bucket-local src), 0 pad
      bgidx [B_SLOTS] int16   phase-B gather idxs (region staging row), 0 pad
      btv   [B_SLOTS] f32     phase-B window-local target value, -1 pad
    """
    lo = core * TPC
    agidx = np.zeros(A_SLOTS, dtype=np.int16)
    bgidx = np.zeros(B_SLOTS, dtype=np.int16)
    btv = np.full(B_SLOTS, -1.0, dtype=np.float32)

    for r in range(NUM_REL):
        src = np.asarray(edge_indices[r, 0])
        tgt = np.asarray(edge_indices[r, 1])
        m = (tgt >= lo) & (tgt < lo + TPC)
        s = src[m].astype(np.int64)
        t = (tgt[m] - lo).astype(np.int64)
        w = t // WIN
        reg = np.minimum(w // WPR, NREG - 1)
        b = s // BROWS

        # ---- phase A placement: group by (region, bucket) ----
        order_a = np.lexsort((t, b, reg))
        rs, bs = reg[order_a], b[order_a]
        grp = rs * NB + bs
        # rank within group
        change = np.empty(grp.shape[0], dtype=bool)
        if grp.shape[0]:
            change[0] = True
            change[1:] = grp[1:] != grp[:-1]
        starts = np.nonzero(change)[0]
        gid = np.cumsum(change) - 1
        rank = np.arange(grp.shape[0]) - starts[gid]
        counts = np.bincount(grp, minlength=NREG * NB)
        if counts.max(initial=0) > SUBCAP:
            raise RuntimeError(
                f"phase-A overflow r{r} core{core}: {counts.max()}")
        # slot space per (rel, bucket): regions in order, SUBCAP each
        aslot = (r * NB + bs) * RB_SLOTS + rs * SUBCAP + rank
        agidx[aslot] = (s[order_a] - bs * BROWS).astype(np.int16)
        # staging row in region table (partition-major permuted within chunk)
        nblk = SUBCAP // 128
        soff = (r * NB + bs) * SUBCAP
        stage_row_sorted = soff + (rank % 128) * nblk + rank // 128
        stage_row = np.empty(t.shape[0], dtype=np.int64)
        stage_row[order_a] = stage_row_sorted

        # ---- phase B placement: group by (window), target-sorted ----
        order_b = np.lexsort((t,))
        wsB, tsB = w[order_b], t[order_b]
        rowsB = stage_row[order_b]
        changeB = np.empty(wsB.shape[0], dtype=bool)
        if wsB.shape[0]:
            changeB[0] = True
            changeB[1:] = wsB[1:] != wsB[:-1]
        startsB = np.nonzero(changeB)[0]
        gidB = np.cumsum(changeB) - 1
        rankB = np.arange(wsB.shape[0]) - startsB[gidB]
        countsB = np.bincount(wsB, minlength=NWIN)
        if countsB.max(initial=0) > BCAP:
            raise RuntimeError(
                f"phase-B overflow r{r} core{core}: {countsB.max()}")
        pair = wsB // 2
        subw = wsB % 2
        bslot = pair * PAIR_SLOTS + (subw * NUM_REL + r) * BCAP + rankB
        bgidx[bslot] = rowsB.astype(np.int16)
        btv[bslot] = (tsB - wsB * WIN).astype(np.float32)
    return agidx, bgidx, btv


def _block_ranges(packs):
    """Per global 128-slot block: union across cores of the active
    window-local target range [lo, hi).  All-pad blocks get (0, 1)."""
    nblocks = B_SLOTS // 128
    btvs = np.stack([p[2] for p in packs])            # [NCORE, B_SLOTS]
    btvs = btvs.reshape(NCORE, nblocks, 128)
    valid = btvs >= 0
    lob = np.where(valid, btvs, np.inf).min(axis=(0, 2))
    hib = np.where(valid, btvs, -np.inf).max(axis=(0, 2))
    ranges = []
    for j in range(nblocks):
        if np.isfinite(lob[j]):
            ranges.append((int(lob[j]), int(hib[j]) + 1))
        else:
            ranges.append((0, 1))
    return ranges


def _wrap16(idx_flat):
    n = idx_flat.shape[0]
    a = idx_flat.reshape(n // 16, 16).T
    return np.tile(a, (8, 1)).copy()


def _slotmaj(v):
    n = v.shape[0]
    return np.ascontiguousarray(v.reshape(n // 128, 128).T)


def _build_program(ranges, queue_map=None):
    import concourse.bacc as bacc
    import concourse.tile as tile
    from concourse import mybir

    nc = bacc.Bacc("TRN2", debug=False, num_swdge_queues=NQUEUE)
    dt = mybir.dt

    xb = [nc.dram_tensor(f"xb{k}", [BROWS, D], dt.bfloat16, kind="ExternalInput")
          for k in range(NB)]
    xto = nc.dram_tensor("xto", [D, TPCP], dt.bfloat16, kind="ExternalInput")
    wt = nc.dram_tensor("wt", [D, 3 * D], dt.bfloat16, kind="ExternalInput")
    agidx_d = nc.dram_tensor("agidx", [128, A_SLOTS // 16], dt.int16,
                             kind="ExternalInput")
    bgidx_d = nc.dram_tensor("bgidx", [128, B_SLOTS // 16], dt.int16,
                             kind="ExternalInput")
    btv_d = nc.dram_tensor("btv", [128, B_SLOTS // 128], dt.float16,
                           kind="ExternalInput")
    iota_d = nc.dram_tensor("iota", [128, WIN], dt.float16, kind="ExternalInput")
    stage = [nc.dram_tensor(f"stage{h}", [STAGE_ROWS, D], dt.bfloat16,
                            kind="ExternalOutput") for h in range(NREG)]
    out_d = nc.dram_tensor("out", [128, TPCP], dt.bfloat16,
                           kind="ExternalOutput")

    nblk_sub = SUBCAP // 128            # 5
    nblk_g = GSLOT // 128               # 8 blocks per gather

    with tile.TileContext(nc) as tc:
        with (
            tc.tile_pool(name="const", bufs=1) as cpool,
            tc.tile_pool(name="ag", bufs=12) as agpool,
            tc.tile_pool(name="bg", bufs=6) as bgpool,
            tc.tile_pool(name="oh", bufs=16) as ohpool,
            tc.tile_pool(name="gsb", bufs=10) as gsbpool,
            tc.tile_pool(name="xt", bufs=4) as xtpool,
            tc.tile_pool(name="osb", bufs=4) as osbpool,
            tc.tile_pool(name="psA", bufs=4, space="PSUM") as psA,
            tc.tile_pool(name="psB", bufs=3, space="PSUM") as psB,
        ):
            wt_sb = cpool.tile([D, 3 * D], dt.bfloat16)
            nc.sync.dma_start(wt_sb[:], wt[:])
            iota_sb = cpool.tile([128, WIN], dt.float16)
            nc.sync.dma_start(iota_sb[:], iota_d[:])
            agidx_sb = cpool.tile([128, A_SLOTS // 16], dt.int16)
            nc.sync.dma_start(agidx_sb[:], agidx_d[:])
            bgidx_sb = cpool.tile([128, B_SLOTS // 16], dt.int16)
            nc.sync.dma_start(bgidx_sb[:], bgidx_d[:])
            btv_sb = cpool.tile([128, B_SLOTS // 128], dt.float16)
            nc.sync.dma_start(btv_sb[:], btv_d[:])

            gcount = [0]
            gather_names = []

            def gq():
                # queue for the next SWDGE gather, from the two-pass map
                q = queue_map[gcount[0]] if queue_map else 0
                gcount[0] += 1
                return q

            def emit_a_level(g, r, b):
                # one 1024-slot gather of (rel, bucket) slot-range
                # [1024g, 1024(g+1)), written to the region tables it spans
                ga = agpool.tile([128, nblk_g, D], dt.bfloat16, tag="ag")
                base = (r * NB + b) * RB_SLOTS
                fb = base + g * GSLOT
                gi = nc.gpsimd.dma_gather(
                    ga[:], xb[b][:],
                    agidx_sb[:, fb // 16:(fb + GSLOT) // 16],
                    GSLOT, GSLOT, D,
                    queue_num=gq(),
                )
                gather_names.append(gi.ins.name)
                soff = (r * NB + b) * SUBCAP
                s0 = g * GSLOT
                s1 = s0 + GSLOT
                h0 = s0 // SUBCAP
                h1 = (s1 - 1) // SUBCAP
                for h in range(h0, h1 + 1):
                    c0 = max(s0, h * SUBCAP) - h * SUBCAP      # chunk-local
                    c1 = min(s1, (h + 1) * SUBCAP) - h * SUBCAP
                    t0b = (h * SUBCAP + c0 - s0) // 128        # tile block
                    nc.sync.dma_start(
                        stage[h][soff:soff + SUBCAP, :].rearrange(
                            "(p j) o -> p j o",
                            p=128)[:, c0 // 128:c1 // 128, :],
                        ga[:, t0b:t0b + (c1 - c0) // 128, :],
                    )

            def emit_b_group(G):
                h = (4 * G) // WPR          # region of this group's windows
                gb = bgpool.tile([128, GROUP_SLOTS // 128, D], dt.bfloat16,
                                 tag="bg")
                goff = G * GROUP_SLOTS
                for g in range(GROUP_SLOTS // GSLOT):
                    fb = goff + g * GSLOT
                    gi = nc.gpsimd.dma_gather(
                        gb[:, g * nblk_g:(g + 1) * nblk_g, :], stage[h][:],
                        bgidx_sb[:, fb // 16:(fb + GSLOT) // 16],
                        GSLOT, GSLOT, D,
                        queue_num=gq(),
                    )
                    gather_names.append(gi.ins.name)
                xt_t = xtpool.tile([D, 4 * WIN], dt.bfloat16, tag="xt")
                nc.sync.dma_start(xt_t[:], xto[:, G * 4 * WIN:(G + 1) * 4 * WIN])
                osb = osbpool.tile([128, 4 * WIN], dt.bfloat16, tag="osb")
                for wl in range(4):
                    wv = 4 * G + wl
                    if wv >= NWIN:
                        continue
                    t0 = wv * WIN
                    outp = psB.tile([128, WIN], dt.float32, tag="psB")
                    nc.tensor.matmul(
                        outp[:], wt_sb[:, 0:D], xt_t[:, wl * WIN:(wl + 1) * WIN],
                        start=True, stop=False,
                    )
                    for r in range(NUM_REL):
                        gps = psA.tile([128, WIN], dt.float32, tag="psA")
                        nc.vector.memset(gps[:], 0.0)
                        jbase = (wl * NUM_REL + r) * nblk_sub
                        for j in range(nblk_sub):
                            bcol = goff // 128 + jbase + j
                            lo, hi = ranges[bcol]
                            oh = ohpool.tile([128, WIN], dt.bfloat16, tag="oh")
                            nc.vector.tensor_tensor(
                                out=oh[:, :hi - lo],
                                in0=btv_sb[:, bcol:bcol + 1]
                                    .to_broadcast([128, hi - lo]),
                                in1=iota_sb[:, lo:hi],
                                op=mybir.AluOpType.is_equal,
                            )
                            nc.tensor.matmul(
                                gps[:, lo:hi],
                                gb[:, jbase + j, :],
                                oh[:, :hi - lo],
                                start=False, stop=(j == nblk_sub - 1),
                                skip_group_check=True,
                            )
                        gsb = gsbpool.tile([128, WIN], dt.bfloat16, tag="gsb")
                        nc.scalar.copy(out=gsb[:], in_=gps[:])
                        nc.tensor.matmul(
                            outp[:], wt_sb[:, (1 + r) * D:(2 + r) * D], gsb[:],
                            start=False, stop=(r == NUM_REL - 1),
                        )
                    nc.scalar.copy(out=osb[:, wl * WIN:(wl + 1) * WIN],
                                   in_=outp[:])
                nc.sync.dma_start(
                    out_d[:, G * 4 * WIN:(G + 1) * 4 * WIN], osb[:])

            # Emission: A levels feed regions in order; B groups of region h
            # need A levels 0..ceil(((h+1)*640)/1024)-1.  Interleave so the
            # SWDGE queues stay busy while PE/DVE consume earlier regions.
            _phase = os.environ.get("KPHASE", "")
            rb = [(r, b) for r in range(NUM_REL) for b in range(NB)]
            nlv = RB_SLOTS // GSLOT                 # 5 A levels
            # B group G (windows 4G..4G+3) is in region G//4; region h is
            # fully staged after A level lv_need[h].  Emit levels 0-1 up
            # front, then interleave later levels' gathers between the B
            # groups they do NOT gate, so the Pool queues and PE both stay
            # busy instead of alternating in bursts.
            lv_need = [((h + 1) * SUBCAP - 1) // GSLOT for h in range(NREG)]
            ready_after = {lv: [] for lv in range(nlv)}
            for G in range(NGROUP):
                ready_after[lv_need[G // 4]].append(G)

            for (r, b) in rb:
                emit_a_level(0, r, b)
            for (r, b) in rb:
                emit_a_level(1, r, b)
            pending_b = ready_after[0] + ready_after[1]
            for lv in range(2, nlv):
                nb_ = len(pending_b)
                na = len(rb)
                k = 0
                for i, G in enumerate(pending_b):
                    if _phase != "A":
                        emit_b_group(G)
                    while k * nb_ < (i + 1) * na:
                        emit_a_level(lv, *rb[k])
                        k += 1
                while k < na:
                    emit_a_level(lv, *rb[k])
                    k += 1
                pending_b = ready_after[lv]
            for G in pending_b:
                if _phase != "A":
                    emit_b_group(G)
    nc.compile()
    return nc, gather_names


def _pool_dma_sched_order(nc):
    """Names of Pool-engine DMA instructions in scheduled (block) order --
    the order the Tile sem-assignment pass walks, which fixes each
    instruction's DMASW sem lane (lane = position % 8)."""
    from concourse import mybir
    order = []
    for f in nc.m.functions:
        for blk in f.blocks:
            for inst in blk.instructions:
                if (getattr(inst, 'engine', None) == mybir.EngineType.Pool
                        and getattr(inst, 'queue_num', None) is not None):
                    order.append(inst.name)
    return order


def _build_with_queues(ranges):
    """Two-pass build: DMASW sem lanes are assigned by scheduled position
    (mod 8), and a sem lane is locked to one SWDGE queue.  queue =
    scheduled_position % NQUEUE satisfies the lock for every lane.  The
    schedule can shift slightly when queue numbers change, so iterate to a
    fixpoint (falling back to single-queue if it doesn't settle)."""
    queue_map = None
    for attempt in range(4):
        nc, names = _build_program(ranges, queue_map)
        sched = {n: i for i, n in enumerate(_pool_dma_sched_order(nc))}
        want = {gi: sched[n] % NQUEUE for gi, n in enumerate(names)}
        if queue_map == want:
            return nc
        queue_map = want
    # last build used the latest map; verify consistency, else rebuild q0
    nc, names = _build_program(ranges, queue_map)
    sched = {n: i for i, n in enumerate(_pool_dma_sched_order(nc))}
    ok = all(sched[n] % NQUEUE == queue_map[gi]
             for gi, n in enumerate(names))
    if ok:
        return nc
    nc, _ = _build_program(ranges, {gi: 0 for gi in range(len(names))})
    return nc


_NC_CACHE = {}


def kernel(x, W0, W1, W_self, edge_indices):
    import ml_dtypes
    from concourse import bass_utils
    from concourse.bass_utils import run_bass_kernel_spmd

    _register_profile_hook()
    bass_utils.upload_artifacts = lambda tmpdir: "local://" + tmpdir

    x = np.asarray(x)
    W0 = np.asarray(W0)
    W1 = np.asarray(W1)
    W_self = np.asarray(W_self)
    edge_indices = np.asarray(edge_indices)

    bf16 = ml_dtypes.bfloat16
    x16 = x.astype(bf16)
    xbufs = [np.ascontiguousarray(x16[k * BROWS:(k + 1) * BROWS])
             for k in range(NB)]
    wt = np.concatenate([W_self.T, W0.T, W1.T], axis=1).astype(bf16)
    iota = np.tile(np.arange(WIN, dtype=np.float16), (128, 1))

    packs = [_pack_core(edge_indices, c) for c in range(NCORE)]
    ranges = _block_ranges(packs)
    if "nc" not in _NC_CACHE:
        _NC_CACHE["nc"] = _build_with_queues(ranges)
    nc = _NC_CACHE["nc"]

    in_maps = []
    for c in range(NCORE):
        agidx, bgidx, btv = packs[c]
        im = {f"xb{k}": xbufs[k] for k in range(NB)}
        xt = np.zeros((D, TPCP), dtype=bf16)
        xt[:, :TPC] = x16[c * TPC:(c + 1) * TPC].T
        im["xto"] = xt
        im["wt"] = wt
        im["agidx"] = _wrap16(agidx)
        im["bgidx"] = _wrap16(bgidx)
        im["btv"] = _slotmaj(btv.astype(np.float16))
        im["iota"] = iota
        in_maps.append(im)

    trace = os.environ.get("KBENCH_TRACE", "0") == "1"
    res = run_bass_kernel_spmd(nc, in_maps, core_ids=list(range(NCORE)),
                               trace=trace)
    if trace:
        print("HW exec time:", res.exec_time_ns, "ns")
        _NC_CACHE["exec_time_ns"] = res.exec_time_ns

    out = np.empty((N, D), dtype=np.float32)
    for c in range(NCORE):
        o = np.asarray(res.results[c]["out"])          # [128, TPCP] bf16
        out[c * TPC:(c + 1) * TPC] = o[:, :TPC].T.astype(np.float32)
    return out
